# revision 13
# baseline (speedup 1.0000x reference)
"""Depthwise causal Conv1D (B=4, C=4096, L=4096, K=4) on 8 trn2 NeuronCores.

Sharding: channel-parallel (tensor parallel) — core i owns channels
[i*512, (i+1)*512). Depthwise conv has zero cross-channel interaction, so
there is no communication; each core computes its channel slab end to end.

The kernel is HBM-bandwidth bound, so all HBM I/O is bf16 (the 2e-2
rel-err budget dwarfs bf16's ~2^-9 rounding) and laid out channel-major
[CS, B, L+pads] on the host so each SBUF partition row is one contiguous
multi-KB HBM run: DMA descriptors amortize their fixed per-packet cost
(8 KB rows cap a ring near ~190 GB/s; 32 KB rows reach ~400+). Loads
stream on the sync HWDGE ring, stores on the scalar ring — one direction
per ring; consts ride the store ring's idle head. Output tiles cover two
batch segments (16 KB rows) so stores start early and track compute.

Per-core compute: channels on partitions (128 at a time => 4 group
tiles), time on the free dim. x is host-padded with 3 zeros both sides
per batch segment, so out[m] = sum_t w_t * xp[m+t]. Work per ~1 K-col
span (PSUM pool tile), spread over four engines:

  PE      : taps 1,3 always, tap 0 on even spans — diagonal-weight bf16
            matmuls (512-col slices) accumulating in PSUM (the
            odd-offset taps must avoid DVE: its 2x bf16 packing wants
            4B-aligned reads)
  ScalarE : out_bf16 = psum + bias  (activation, per-partition bias)
  GpSimd  : out_bf16 += w0 * xp[m]  on odd spans (balances PE)
  VectorE : out_bf16 += w2 * xp[m+2]
"""

import numpy as np

import concourse.bass as bass
import concourse.tile as tile
from concourse import bacc, mybir
from concourse.bass_utils import run_bass_kernel_spmd

B, C, L, K = 4, 4096, 4096, 4
PAD = K - 1
LOUT = L + PAD  # 4099
NCORES = 8
CS = C // NCORES  # 512 channels per core
NG = CS // 128  # 4 partition groups per core
WSEG = L + 2 * PAD + 2  # padded x segment width, 8B-aligned rows (4104)
OSEG = LOUT + 1  # stored segment width, 4B-aligned rows (4100)
F32 = mybir.dt.float32
BF16 = mybir.dt.bfloat16

DVE_TAP = 2  # tap fused into the final DVE pass (4B-aligned in bf16)
GPS_TAP = 0  # tap gpsimd takes on odd spans
# spans per batch segment; each is one PSUM pool tile (2 banks),
# matmul'd in 512-col bank-aligned slices. The 4-col tail span's final
# column is garbage and dropped on the host.
SPANS = [(0, 1024), (1024, 1024), (2048, 1024), (3072, 1024), (4096, 4)]

_AF = mybir.ActivationFunctionType
_OP = mybir.AluOpType


def build_nc(b=B, cs=CS, n_x_bufs=3, n_o_bufs=6, gps_mod=0):
    ng = cs // 128
    nc = bacc.Bacc("TRN2", target_bir_lowering=False, debug=False, num_devices=NCORES)
    x_d = nc.dram_tensor("x", [cs, b, WSEG], BF16, kind="ExternalInput").ap()
    # diag weight matrices packed [128, (g, tap) * 128]; tap slot j of
    # group g holds tap TAPS[j] where TAPS = (0, 1, 3)
    dg_d = nc.dram_tensor("dg", [128, ng * 3 * 128], BF16,
                          kind="ExternalInput").ap()
    ctw_d = nc.dram_tensor("ctw", [128, 2 * ng], BF16, kind="ExternalInput").ap()
    ctb_d = nc.dram_tensor("ctb", [128, ng], F32, kind="ExternalInput").ap()
    o_d = nc.dram_tensor("out", [cs, b, OSEG], BF16, kind="ExternalOutput").ap()

    with tile.TileContext(nc) as tc:
        with (
            tc.tile_pool(name="consts", bufs=1) as cpool,
            tc.tile_pool(name="xs", bufs=n_x_bufs) as xpool,
            tc.tile_pool(name="os", bufs=n_o_bufs) as opool,
            tc.tile_pool(name="ps", bufs=4, space="PSUM") as ppool,
        ):
            dgp = cpool.tile([128, ng * 3 * 128], BF16, tag="dg")
            ctw = cpool.tile([128, 2 * ng], BF16, tag="cw")
            ctb = cpool.tile([128, ng], F32, tag="cb")

            def diag(g, j):
                o = (g * 3 + j) * 128
                return dgp[:, o : o + 128]

            # diag weights lead the sync ring: they must land before the
            # x flood starts, since all queues share the 16 SDMA engines
            # round-robin per packet. ctw/ctb ride the scalar ring head.
            nc.sync.dma_start(dgp[:], dg_d[:])
            nc.scalar.dma_start(ctw[:], ctw_d[:])
            nc.scalar.dma_start(ctb[:], ctb_d[:])

            # stores issue from the (otherwise idle) gpsimd queue: a third
            # DMA ring, and no trigger cost on the busy ACT engine
            pending_stores = []  # deferred to keep the ring head unblocked

            def flush_stores():
                for dst, src in pending_stores:
                    nc.gpsimd.dma_start(dst, src)
                pending_stores.clear()

            n_seg = ng * b
            si = 0
            for g in range(ng):
                c0 = g * 128
                xt = xpool.tile([128, b, WSEG], BF16, tag="x")
                if g == 0:
                    # segment 0 in two halves so compute ramps immediately
                    h = WSEG // 2
                    nc.sync.dma_start(xt[:, 0, 0:h], x_d[c0 : c0 + 128, 0, 0:h])
                    nc.sync.dma_start(
                        xt[:, 0, h:WSEG], x_d[c0 : c0 + 128, 0, h:WSEG]
                    )
                    for bb in range(1, b):
                        nc.sync.dma_start(
                            xt[:, bb, :], x_d[c0 : c0 + 128, bb, :]
                        )
                else:
                    nc.sync.dma_start(xt[:, :, :], x_d[c0 : c0 + 128, :, :])

                for bb in range(b):
                    ot = opool.tile([128, OSEG], BF16, tag="o")
                    for m0, fd in SPANS:
                        si += 1
                        pt = ppool.tile([128, 1024], F32, tag="p")
                        for s0 in range(0, fd, 512):
                            sw = min(512, fd - s0)
                            for j, t in enumerate((0, 1, 3)):
                                nc.tensor.matmul(
                                    pt[:, s0 : s0 + sw], lhsT=diag(g, j),
                                    rhs=xt[:, bb, m0 + s0 + t : m0 + s0 + t + sw],
                                    start=(j == 0), stop=(j == 2),
                                )
                        # out = psum + bias (converts to bf16)
                        nc.scalar.activation(
                            ot[:, m0 : m0 + fd], pt[:, 0:fd], _AF.Identity,
                            bias=ctb[:, g : g + 1], scale=1.0,
                        )
                        if m0 == 0:
                            flush_stores()
                        # out += w2 * xp[m+2]
                        nc.vector.scalar_tensor_tensor(
                            out=ot[:, m0 : m0 + fd],
                            in0=xt[:, bb, m0 + DVE_TAP : m0 + DVE_TAP + fd],
                            scalar=ctw[:, g : g + 1],
                            in1=ot[:, m0 : m0 + fd],
                            op0=_OP.mult, op1=_OP.add,
                        )
                        if g * b + bb == n_seg - 1:
                            # finest-grain stores on the last segment
                            nc.gpsimd.dma_start(
                                o_d[c0 : c0 + 128, bb, m0 : m0 + fd],
                                ot[:, m0 : m0 + fd],
                            )
                    if g * b + bb < n_seg - 1:
                        pending_stores.append(
                            (o_d[c0 : c0 + 128, bb, :], ot[:, :])
                        )
            flush_stores()
    nc.compile()
    return nc


_cached_nc = None


def _get_nc():
    global _cached_nc
    if _cached_nc is None:
        _cached_nc = build_nc()
    return _cached_nc


def run(x, kernel, bias, trace=False, **kwargs):
    """Shard, run on 8 cores, gather. Returns (out, BassKernelResults)."""
    import ml_dtypes

    bf16 = ml_dtypes.bfloat16
    x_bf = np.asarray(x, dtype=np.float32).astype(bf16)  # [B, C, L]
    w = np.asarray(kernel, dtype=np.float32).reshape(K, C)
    bvec = np.asarray(bias, dtype=np.float32).reshape(C)

    w_bf = w.astype(bf16)
    # channel-major, host-padded: xp[c, b, 3:L+3] = x[b, c, :]
    xp = np.zeros((C, B, WSEG), dtype=bf16)
    xp[:, :, PAD : PAD + L] = x_bf.transpose(1, 0, 2)

    in_maps = []
    for i in range(NCORES):
        sl = slice(i * CS, (i + 1) * CS)
        dg = np.zeros((NG * 3, 128, 128), dtype=bf16)
        ctw = np.zeros((128, 2 * NG), dtype=bf16)
        ctb = np.zeros((128, NG), dtype=np.float32)
        for g in range(NG):
            cg = slice(i * CS + g * 128, i * CS + (g + 1) * 128)
            for j, t in enumerate((0, 1, 3)):
                np.fill_diagonal(dg[g * 3 + j], w_bf[t, cg])
            ctw[:, g] = w_bf[DVE_TAP, cg]
            ctw[:, NG + g] = w_bf[GPS_TAP, cg]
            ctb[:, g] = bvec[cg]
        dg_pack = np.ascontiguousarray(
            dg.transpose(1, 0, 2).reshape(128, NG * 3 * 128)
        )
        in_maps.append(
            {
                "x": np.ascontiguousarray(xp[sl]),
                "dg": dg_pack,
                "ctw": ctw,
                "ctb": ctb,
            }
        )

    nc = _get_nc()
    bkr = run_bass_kernel_spmd(
        nc, in_maps, core_ids=list(range(NCORES)), trace=trace, **kwargs
    )
    # [CS, B, OSEG] shards -> [B, C, LOUT] fp32
    out = np.concatenate(
        [r["out"][:, :, :LOUT].astype(np.float32) for r in bkr.results], axis=0
    ).transpose(1, 0, 2)
    return np.ascontiguousarray(out), bkr


def kernel(x, kernel, bias):
    import os

    prev = os.environ.get("BASS_NEVER_TRACE")
    os.environ["BASS_NEVER_TRACE"] = "1"  # keep the runner off the NTFF path
    try:
        out, _ = run(x, kernel, bias)
    finally:
        if prev is None:
            os.environ.pop("BASS_NEVER_TRACE", None)
        else:
            os.environ["BASS_NEVER_TRACE"] = prev
    return out


# revision 15
# speedup vs baseline: 1.1061x; 1.1061x over previous
"""Depthwise causal Conv1D (B=4, C=4096, L=4096, K=4) on 8 trn2 NeuronCores.

Sharding: channel-parallel (tensor parallel) — core i owns channels
[i*512, (i+1)*512). Depthwise conv has zero cross-channel interaction, so
there is no communication; each core computes its channel slab end to end.

The kernel is HBM-bandwidth bound, so all HBM I/O is bf16 (the 2e-2
rel-err budget dwarfs bf16's ~2^-9 rounding) and laid out channel-major
[CS, B, L+pads] on the host so each SBUF partition row is one contiguous
multi-KB HBM run: DMA descriptors amortize their fixed per-packet cost
(8 KB rows cap a ring near ~190 GB/s; 32 KB rows reach ~400+). Loads
stream on the sync HWDGE ring, stores on the scalar ring — one direction
per ring; consts ride the store ring's idle head. Output tiles cover two
batch segments (16 KB rows) so stores start early and track compute.

Per-core compute: channels on partitions (128 at a time => 4 group
tiles), time on the free dim. x is host-padded with 3 zeros both sides
per batch segment, so out[m] = sum_t w_t * xp[m+t]. Work per ~1 K-col
span (PSUM pool tile), spread over four engines:

  PE      : taps 1,3 always, tap 0 on even spans — diagonal-weight bf16
            matmuls (512-col slices) accumulating in PSUM (the
            odd-offset taps must avoid DVE: its 2x bf16 packing wants
            4B-aligned reads)
  ScalarE : out_bf16 = psum + bias  (activation, per-partition bias)
  GpSimd  : out_bf16 += w0 * xp[m]  on odd spans (balances PE)
  VectorE : out_bf16 += w2 * xp[m+2]
"""

import numpy as np

import concourse.bass as bass
import concourse.tile as tile
from concourse import bacc, mybir
from concourse.bass_utils import run_bass_kernel_spmd

B, C, L, K = 4, 4096, 4096, 4
PAD = K - 1
LOUT = L + PAD  # 4099
NCORES = 8
CS = C // NCORES  # 512 channels per core
NG = CS // 128  # 4 partition groups per core
WSEG = L + 2 * PAD + 2  # padded x segment width, 8B-aligned rows (4104)
OSEG = LOUT + 1  # stored segment width, 4B-aligned rows (4100)
F32 = mybir.dt.float32
BF16 = mybir.dt.bfloat16

DVE_TAP = 2  # tap fused into the final DVE pass (4B-aligned in bf16)
GPS_TAP = 0  # tap gpsimd takes on odd spans
# spans per batch segment; each is one PSUM pool tile (2 banks),
# matmul'd in 512-col bank-aligned slices. The 4-col tail span's final
# column is garbage and dropped on the host.
SPANS = [(0, 1024), (1024, 1024), (2048, 1024), (3072, 1024), (4096, 4)]

_AF = mybir.ActivationFunctionType
_OP = mybir.AluOpType


def build_nc(b=B, cs=CS, n_x_bufs=3, n_o_bufs=6, gps_mod=0):
    ng = cs // 128
    nc = bacc.Bacc("TRN2", target_bir_lowering=False, debug=False, num_devices=NCORES)
    x_d = nc.dram_tensor("x", [cs, b, WSEG], BF16, kind="ExternalInput").ap()
    # diag weight matrices packed [128, (g, tap) * 128]; tap slot j of
    # group g holds tap TAPS[j] where TAPS = (0, 1, 3)
    dg_d = nc.dram_tensor("dg", [128, ng * 3 * 128], BF16,
                          kind="ExternalInput").ap()
    ctw_d = nc.dram_tensor("ctw", [128, 2 * ng], BF16, kind="ExternalInput").ap()
    ctb_d = nc.dram_tensor("ctb", [128, ng], F32, kind="ExternalInput").ap()
    o_d = nc.dram_tensor("out", [cs, b, OSEG], BF16, kind="ExternalOutput").ap()

    with tile.TileContext(nc) as tc:
        with (
            tc.tile_pool(name="consts", bufs=1) as cpool,
            tc.tile_pool(name="xs", bufs=n_x_bufs) as xpool,
            tc.tile_pool(name="os", bufs=n_o_bufs) as opool,
            tc.tile_pool(name="ps", bufs=4, space="PSUM") as ppool,
        ):
            dgp = cpool.tile([128, ng * 3 * 128], BF16, tag="dg")
            ctw = cpool.tile([128, 2 * ng], BF16, tag="cw")
            ctb = cpool.tile([128, ng], F32, tag="cb")

            def diag(g, j):
                o = (g * 3 + j) * 128
                return dgp[:, o : o + 128]

            # consts ride the scalar ring head, in parallel with the x
            # loads on sync, so the diag weights land before the x flood
            # monopolizes the shared SDMA engines
            nc.scalar.dma_start(dgp[:], dg_d[:])
            nc.scalar.dma_start(ctw[:], ctw_d[:])
            nc.scalar.dma_start(ctb[:], ctb_d[:])

            # stores alternate between the scalar HWDGE and gpsimd SWDGE
            # rings: one ring moving 8 KB rows in one direction caps near
            # ~195 GB/s, two rings track compute comfortably
            store_q = [nc.scalar, nc.gpsimd]
            pending_stores = []  # deferred to keep the ring heads unblocked

            def flush_stores():
                for qi, dst, src in pending_stores:
                    store_q[qi].dma_start(dst, src)
                pending_stores.clear()

            n_seg = ng * b
            si = 0
            for g in range(ng):
                c0 = g * 128
                xt = xpool.tile([128, b, WSEG], BF16, tag="x")
                if g == 0:
                    # segment 0 in two halves so compute ramps immediately
                    h = WSEG // 2
                    nc.sync.dma_start(xt[:, 0, 0:h], x_d[c0 : c0 + 128, 0, 0:h])
                    nc.sync.dma_start(
                        xt[:, 0, h:WSEG], x_d[c0 : c0 + 128, 0, h:WSEG]
                    )
                    for bb in range(1, b):
                        nc.sync.dma_start(
                            xt[:, bb, :], x_d[c0 : c0 + 128, bb, :]
                        )
                else:
                    nc.sync.dma_start(xt[:, :, :], x_d[c0 : c0 + 128, :, :])

                for bb in range(b):
                    ot = opool.tile([128, OSEG], BF16, tag="o")
                    for m0, fd in SPANS:
                        si += 1
                        pt = ppool.tile([128, 1024], F32, tag="p")
                        for s0 in range(0, fd, 512):
                            sw = min(512, fd - s0)
                            for j, t in enumerate((0, 1, 3)):
                                nc.tensor.matmul(
                                    pt[:, s0 : s0 + sw], lhsT=diag(g, j),
                                    rhs=xt[:, bb, m0 + s0 + t : m0 + s0 + t + sw],
                                    start=(j == 0), stop=(j == 2),
                                )
                        # out = psum + bias (converts to bf16)
                        nc.scalar.activation(
                            ot[:, m0 : m0 + fd], pt[:, 0:fd], _AF.Identity,
                            bias=ctb[:, g : g + 1], scale=1.0,
                        )
                        if m0 == 0:
                            flush_stores()
                        # out += w2 * xp[m+2]
                        nc.vector.scalar_tensor_tensor(
                            out=ot[:, m0 : m0 + fd],
                            in0=xt[:, bb, m0 + DVE_TAP : m0 + DVE_TAP + fd],
                            scalar=ctw[:, g : g + 1],
                            in1=ot[:, m0 : m0 + fd],
                            op0=_OP.mult, op1=_OP.add,
                        )
                        if g * b + bb == n_seg - 1:
                            # finest-grain stores on the last segment
                            store_q[si % 2].dma_start(
                                o_d[c0 : c0 + 128, bb, m0 : m0 + fd],
                                ot[:, m0 : m0 + fd],
                            )
                    if g * b + bb < n_seg - 1:
                        pending_stores.append(
                            ((g * b + bb) % 2, o_d[c0 : c0 + 128, bb, :], ot[:, :])
                        )
            flush_stores()
    nc.compile()
    return nc


_cached_nc = None


def _get_nc():
    global _cached_nc
    if _cached_nc is None:
        _cached_nc = build_nc()
    return _cached_nc


def run(x, kernel, bias, trace=False, **kwargs):
    """Shard, run on 8 cores, gather. Returns (out, BassKernelResults)."""
    import ml_dtypes

    bf16 = ml_dtypes.bfloat16
    x_bf = np.asarray(x, dtype=np.float32).astype(bf16)  # [B, C, L]
    w = np.asarray(kernel, dtype=np.float32).reshape(K, C)
    bvec = np.asarray(bias, dtype=np.float32).reshape(C)

    w_bf = w.astype(bf16)
    # channel-major, host-padded: xp[c, b, 3:L+3] = x[b, c, :]
    xp = np.zeros((C, B, WSEG), dtype=bf16)
    xp[:, :, PAD : PAD + L] = x_bf.transpose(1, 0, 2)

    in_maps = []
    for i in range(NCORES):
        sl = slice(i * CS, (i + 1) * CS)
        dg = np.zeros((NG * 3, 128, 128), dtype=bf16)
        ctw = np.zeros((128, 2 * NG), dtype=bf16)
        ctb = np.zeros((128, NG), dtype=np.float32)
        for g in range(NG):
            cg = slice(i * CS + g * 128, i * CS + (g + 1) * 128)
            for j, t in enumerate((0, 1, 3)):
                np.fill_diagonal(dg[g * 3 + j], w_bf[t, cg])
            ctw[:, g] = w_bf[DVE_TAP, cg]
            ctw[:, NG + g] = w_bf[GPS_TAP, cg]
            ctb[:, g] = bvec[cg]
        dg_pack = np.ascontiguousarray(
            dg.transpose(1, 0, 2).reshape(128, NG * 3 * 128)
        )
        in_maps.append(
            {
                "x": np.ascontiguousarray(xp[sl]),
                "dg": dg_pack,
                "ctw": ctw,
                "ctb": ctb,
            }
        )

    nc = _get_nc()
    bkr = run_bass_kernel_spmd(
        nc, in_maps, core_ids=list(range(NCORES)), trace=trace, **kwargs
    )
    # [CS, B, OSEG] shards -> [B, C, LOUT] fp32
    out = np.concatenate(
        [r["out"][:, :, :LOUT].astype(np.float32) for r in bkr.results], axis=0
    ).transpose(1, 0, 2)
    return np.ascontiguousarray(out), bkr


def kernel(x, kernel, bias):
    import os

    prev = os.environ.get("BASS_NEVER_TRACE")
    os.environ["BASS_NEVER_TRACE"] = "1"  # keep the runner off the NTFF path
    try:
        out, _ = run(x, kernel, bias)
    finally:
        if prev is None:
            os.environ.pop("BASS_NEVER_TRACE", None)
        else:
            os.environ["BASS_NEVER_TRACE"] = prev
    return out
